# revision 3
# baseline (speedup 1.0000x reference)
"""Sparse-attention Trainium2 kernel (nn_Attention_44341242364527).

Strategy
--------
Head-tensor-parallel over 8 NeuronCores (2 heads/core, Megatron-style:
Wq/Wk/Wv column-sharded, Wo row-sharded, partial outputs all-reduced on
the host during unshard).

The sparse gather ``k[idx]`` / ``v[idx]`` is reformulated densely: since
``exp(qk/sqrt(D) + geo) = exp(qk/sqrt(D)) * exp(geo)``, and idx/valid/
geo_bias are host-known inputs, the host pre-scatters

    WT[h][s', s] = sum_k 1[idx[s,k]==s' & valid & s'<=s] * exp(geo[h,s,k])

Then per head, on device (everything transposed so no on-chip transposes
are needed):

    ST  = Kh @ Qh.T                  [s', s]   (dense scores)
    AT  = exp(ST/sqrt(D)) * WT       [s', s]   (un-normalized attention)
    AOT = Vh.T @ AT                  [d, s]    (un-normalized context)
    Z   = ones @ AT                  [1, s]    (softmax denominator)
    Y  += (AOT/Z).T @ WoT_shard      [s, HID]  (partial output)

Causality makes AT block-lower-triangular: only ~62% of blocks are
computed. WT==0 kills both the masked and the un-selected entries.
"""

import math
import sys

sys.path.insert(0, "/opt/trn_rl_repo")

import numpy as np

B, S, H, D, KS = 1, 2048, 16, 128, 64
HID = H * D
NCORES = 8
HPC = H // NCORES          # heads per core
CPC = HPC * D              # output channels per core
P = 128                    # partitions
SC = 512                   # s-chunk (PSUM bank width in f32)
NJ = S // SC               # 4 s-chunks
NT = S // P                # 16 s'-tiles
NK = HID // P              # 16 contraction chunks

# dtype knobs (numpy dtype name per tensor class); PSUM is always f32.
DT_PROJ = "float32"        # hsT + Wq/Wk/Wv operands of the QKV projections
DT_QK = "float32"          # Q^T/K^T operands of the score matmul
DT_ATT = "float32"         # exp(S)*W and V operands of the AV matmul
DT_WT = "float32"          # the scattered exp(geo) tensor (DMA-heavy)
DT_WO = "float32"          # AOT and Wo operands of the output projection

_CACHE = {}


def _np_dt(name):
    if name == "bfloat16":
        import ml_dtypes

        return np.dtype(ml_dtypes.bfloat16)
    return np.dtype(name)


def _my_dt(name):
    from concourse import mybir

    return {
        "float32": mybir.dt.float32,
        "bfloat16": mybir.dt.bfloat16,
    }[name]


def _build_nc():
    import concourse.tile as tile
    from concourse import bacc, mybir

    F32 = mybir.dt.float32
    EXP = mybir.ActivationFunctionType.Exp
    MULT = mybir.AluOpType.mult

    nc = bacc.Bacc("TRN2", target_bir_lowering=False, debug=False,
                   num_devices=NCORES)

    hsT = nc.dram_tensor("hsT", [HID, S], _my_dt(DT_PROJ), kind="ExternalInput")
    wqT = nc.dram_tensor("wqT", [HID, CPC], _my_dt(DT_PROJ), kind="ExternalInput")
    wkT = nc.dram_tensor("wkT", [HID, CPC], _my_dt(DT_PROJ), kind="ExternalInput")
    wvT = nc.dram_tensor("wvT", [HID, CPC], _my_dt(DT_PROJ), kind="ExternalInput")
    woT = nc.dram_tensor("woT", [CPC, HID], _my_dt(DT_WO), kind="ExternalInput")
    wt = nc.dram_tensor("wt", [HPC, S, S], _my_dt(DT_WT), kind="ExternalInput")
    y = nc.dram_tensor("y", [S, HID], F32, kind="ExternalOutput")

    inv_sqrt_d = 1.0 / math.sqrt(D)

    with tile.TileContext(nc) as tc:
        with tc.tile_pool(name="persist", bufs=1) as persist:
            QT = [persist.tile([P, S], _my_dt(DT_QK), tag=f"qt{h}", name=f"qt{h}")
                  for h in range(HPC)]
            KT = [persist.tile([P, S], _my_dt(DT_QK), tag=f"kt{h}", name=f"kt{h}")
                  for h in range(HPC)]
            Vsb = [persist.tile([P, CPC], _my_dt(DT_ATT), tag=f"v{t}", name=f"vres{t}")
                   for t in range(NT)]
            AOT = [persist.tile([P, S], _my_dt(DT_WO), tag=f"aot{h}", name=f"aot{h}")
                   for h in range(HPC)]
            ones_col = persist.tile([P, 1], _my_dt(DT_ATT), tag="ones_col", name="ones_col")
            ones_row = persist.tile([1, P], F32, tag="ones_row", name="ones_row")
            nc.gpsimd.memset(ones_col[:], 1.0)
            nc.gpsimd.memset(ones_row[:], 1.0)

            # ---- Phase 1: QKV projections -> QT/KT [d, s], V [s, d] ----
            with tc.tile_pool(name="wpool", bufs=1) as wpool, \
                 tc.tile_pool(name="hpool", bufs=32) as hpool, \
                 tc.tile_pool(name="ps1", bufs=1, space="PSUM") as ps1:
                wq_sb, wk_sb, wv_sb = [], [], []
                for k in range(NK):
                    for lst, dram, nm in ((wq_sb, wqT, "wq"), (wk_sb, wkT, "wk"),
                                          (wv_sb, wvT, "wv")):
                        t_ = wpool.tile([P, CPC], _my_dt(DT_PROJ), tag=f"{nm}{k}", name=f"{nm}{k}")
                        nc.sync.dma_start(t_[:], dram[k * P:(k + 1) * P, :])
                        lst.append(t_)

                for j in range(NJ):
                    hs_t = []
                    for k in range(NK):
                        t_ = hpool.tile([P, SC], _my_dt(DT_PROJ), tag="hst", name="hst")
                        nc.sync.dma_start(
                            t_[:], hsT[k * P:(k + 1) * P, j * SC:(j + 1) * SC])
                        hs_t.append(t_)
                    for h in range(HPC):
                        for w_sb, acc, nm in ((wq_sb, QT, "q"), (wk_sb, KT, "k")):
                            pp = ps1.tile([P, SC], F32, tag=f"{nm}{h}", name=f"ps_{nm}{h}")
                            for k in range(NK):
                                nc.tensor.matmul(
                                    pp[:], w_sb[k][:, h * D:(h + 1) * D],
                                    hs_t[k][:],
                                    start=(k == 0), stop=(k == NK - 1))
                            nc.vector.tensor_copy(
                                acc[h][:, j * SC:(j + 1) * SC], pp[:])
                    for si in range(SC // P):
                        vp = ps1.tile([P, CPC], F32, tag=f"v{si}", name=f"ps_v{si}")
                        for k in range(NK):
                            nc.tensor.matmul(
                                vp[:], hs_t[k][:, si * P:(si + 1) * P],
                                wv_sb[k][:],
                                start=(k == 0), stop=(k == NK - 1))
                        nc.vector.tensor_copy(Vsb[4 * j + si][:], vp[:])

            # ---- Phase 2: attention (transposed, causally blocked) ----
            with tc.tile_pool(name="wtp", bufs=4) as wtp, \
                 tc.tile_pool(name="atp", bufs=4) as atp, \
                 tc.tile_pool(name="smp", bufs=2) as smp, \
                 tc.tile_pool(name="ps2", bufs=2, space="PSUM") as ps2, \
                 tc.tile_pool(name="ps2a", bufs=2, space="PSUM") as ps2a:
                for h in range(HPC):
                    for j in range(NJ):
                        aop = ps2a.tile([P, SC], F32, tag="ao", name="ao")
                        zp = ps2a.tile([1, SC], F32, tag="z", name="z")
                        tmax = min(4 * j + 3, NT - 1)
                        for t in range(tmax + 1):
                            stp = ps2.tile([P, SC], F32, tag="st", name="st")
                            nc.tensor.matmul(
                                stp[:], KT[h][:, t * P:(t + 1) * P],
                                QT[h][:, j * SC:(j + 1) * SC],
                                start=True, stop=True)
                            at = atp.tile([P, SC], _my_dt(DT_ATT), tag="at", name="at")
                            nc.scalar.activation(at[:], stp[:], EXP,
                                                 scale=inv_sqrt_d)
                            wt_sb = wtp.tile([P, SC], _my_dt(DT_WT), tag="wt", name="wt")
                            nc.sync.dma_start(
                                wt_sb[:],
                                wt[h, t * P:(t + 1) * P, j * SC:(j + 1) * SC])
                            nc.vector.tensor_mul(at[:], at[:], wt_sb[:])
                            nc.tensor.matmul(
                                aop[:], Vsb[t][:, h * D:(h + 1) * D], at[:],
                                start=(t == 0), stop=(t == tmax))
                            nc.tensor.matmul(
                                zp[:], ones_col[:], at[:],
                                start=(t == 0), stop=(t == tmax))
                        r = smp.tile([1, SC], F32, tag="r", name="r")
                        nc.vector.reciprocal(r[:], zp[:])
                        rb = ps2.tile([P, SC], F32, tag="rb", name="rb")
                        nc.tensor.matmul(rb[:], ones_row[:], r[:],
                                         start=True, stop=True)
                        rbs = atp.tile([P, SC], F32, tag="rbs", name="rbs")
                        nc.scalar.copy(rbs[:], rb[:])
                        nc.vector.tensor_tensor(
                            AOT[h][:, j * SC:(j + 1) * SC], aop[:], rbs[:],
                            MULT)

            # ---- Phase 3: output projection (partial; host adds bias) ----
            with tc.tile_pool(name="wop", bufs=1) as wop, \
                 tc.tile_pool(name="ypool", bufs=4) as ypool, \
                 tc.tile_pool(name="ps3", bufs=2, space="PSUM") as ps3:
                wo_sb = []
                for h in range(HPC):
                    t_ = wop.tile([P, HID], _my_dt(DT_WO), tag=f"wo{h}", name=f"wo{h}")
                    nc.sync.dma_start(t_[:], woT[h * P:(h + 1) * P, :])
                    wo_sb.append(t_)
                for m in range(NT):
                    for n in range(NJ):
                        yps = ps3.tile([P, SC], F32, tag="y", name="ps_y")
                        for h in range(HPC):
                            nc.tensor.matmul(
                                yps[:], AOT[h][:, m * P:(m + 1) * P],
                                wo_sb[h][:, n * SC:(n + 1) * SC],
                                start=(h == 0), stop=(h == HPC - 1))
                        ysb = ypool.tile([P, SC], F32, tag="ysb", name="ysb")
                        nc.vector.tensor_copy(ysb[:], yps[:])
                        nc.sync.dma_start(
                            y[m * P:(m + 1) * P, n * SC:(n + 1) * SC], ysb[:])

    nc.compile()
    return nc


def _get_nc():
    if "nc" not in _CACHE:
        _CACHE["nc"] = _build_nc()
    return _CACHE["nc"]


def make_in_maps(hidden_states, idx, valid, geo_bias, Wq, Wk, Wv, Wo):
    """Host-side sharding/layout prep: returns the 8 per-core input maps."""
    hs = np.ascontiguousarray(np.asarray(hidden_states, np.float32)[0])
    idx = np.asarray(idx).astype(np.int64)
    valid = np.asarray(valid).astype(bool)
    geo = np.asarray(geo_bias, np.float32)

    dt_proj, dt_wo, dt_wt = _np_dt(DT_PROJ), _np_dt(DT_WO), _np_dt(DT_WT)

    hsT = np.ascontiguousarray(hs.T).astype(dt_proj)       # [HID, S]

    srange = np.arange(S)
    cmask = ((idx <= srange[:, None]) & valid).ravel()
    flat = (idx * S + srange[:, None]).ravel()[cmask]
    eg = np.exp(np.asarray(geo_bias, np.float64))          # [H, S, K]

    in_maps = []
    for c in range(NCORES):
        h0 = HPC * c
        sl = slice(h0 * D, (h0 + HPC) * D)
        wt_c = np.empty((HPC, S, S), dt_wt)
        for hh in range(HPC):
            wt_c[hh] = (np.bincount(flat,
                                    weights=eg[h0 + hh].ravel()[cmask],
                                    minlength=S * S)
                        .reshape(S, S).astype(dt_wt))
        in_maps.append({
            "hsT": hsT,
            "wqT": np.ascontiguousarray(np.asarray(Wq)[sl].T).astype(dt_proj),
            "wkT": np.ascontiguousarray(np.asarray(Wk)[sl].T).astype(dt_proj),
            "wvT": np.ascontiguousarray(np.asarray(Wv)[sl].T).astype(dt_proj),
            "woT": np.ascontiguousarray(np.asarray(Wo)[:, sl].T).astype(dt_wo),
            "wt": wt_c,
        })
    return in_maps


def kernel(hidden_states, idx, valid, geo_bias, Wq, Wk, Wv, Wo, bo):
    from concourse import bass_utils

    nc = _get_nc()
    in_maps = make_in_maps(hidden_states, idx, valid, geo_bias, Wq, Wk, Wv, Wo)
    res = bass_utils.run_bass_kernel_spmd(nc, in_maps,
                                          core_ids=list(range(NCORES)))
    out = np.zeros((S, HID), np.float32)
    for r in res.results:
        out += r["y"]
    out += np.asarray(bo, np.float32)
    return out.reshape(B, S, HID)
